# revision 63
# baseline (speedup 1.0000x reference)
"""BlanchotianAttention TRN2 kernel: 8 NeuronCores, data-parallel over batch (2)
x tensor-parallel over heads (4 heads/core).

Final design (cost-model-driven; ACT/exp is the bottleneck engine ~139us
busy; everything else is scheduled to keep it fed):
  - inputs xT/wqkv/wout shipped bf16 (halves DMA). Column-grouped DMAs (one
    instruction per logical block spanning all k-tiles); wqkv host column
    order [q-p0 | k-p0 | q-p1 | k-p1 | v] so each pair's q+k arrive in one
    contiguous transfer. 12 dummy matmuls ramp the PE p-state to full clock
    under the DMA window (full speed needs 3us of continuous execution).
  - stage A (bf16): qkvT = w.T @ xT; q/k columns pre-scaled on host by
    sqrt(dim^-0.5 / temperature_h) (split so fp8 quantization of BOTH q and
    k stays in e4m3's normal range). PSUM accumulators live on the spare
    PSUM bank, NOT on the score-tile rotation, so interleaved stage-A work
    never delays the exp feed; av pairs share one bank (one start marks the
    bank, the sibling region lands on the pending-zero). Evacuation casts
    q/k to fp8 (qk8) and v to bf16 (va65) directly. Deferred A chunks are
    spread one-per-jt over ic0 by deadline (k cols of group g before
    scores(0, 4g); av(st) before PV(0, st)).
  - scores (fp8 DoubleRow, 0.5 cycles/row): per (ic, jt, head) one DR matmul
    lhsT = k8[d 64, 2 planes, j 128], rhs = q8[d 64, 2 planes, i 512] ->
    S^T [128 j, 512 i]; plane 1 is zeroed once so the second k-tile of the
    DR pair contributes nothing.
  - exp on ACT -> P bf16 [128 j, 1024 (2 heads x 512 i)] in SBUF.
  - PV in [i, d] orientation: pvl[128 i, 65] += P_blk^T @ [v_h | ones]
    (65-wide output = cheap in the out-free-dim-priced cost model, and all
    128 out partitions useful; the 65th column accumulates the softmax
    denominator l). 16 (i-block, head) regions packed 7-per-bank in a 3-bank
    PSUM tile; one start=True per bank, other regions ride the bank-wide
    pending-zero mark (PSUM rule: a second start in a bank clobbers it).
  - void token (j = 2048) has no j-tile: tiny block-diagonal fp8 matmuls
    give s_void in [i, head] orientation ([128 i, 2] outputs), one 16-col
    exp per i-chunk, and the rank-1 p_void x [v_void | 1] term is folded
    in-place (scalar_tensor_tensor) into the pvl->SBUF evacuation.
  - normalize: pvl -> SBUF fat copy (so the next ic's PV never waits),
    reciprocal of l columns, per-partition-scalar multiply -> O_norm bf16;
    PE-transpose (bf16 identity ifmap) back to [d, i]; evacuate to osb.
  - stage D: y = osb.T @ w_out (bf16), psum -> ysb -> per-half DMA out; host
    sums the 4 head-group partials per batch (+ b_out).
  - tail: the last i-chunk's norm/transpose/outproj chain runs in four
    parallel lanes on the four freed PSUM tag-slots, stage-interleaved so
    the per-engine in-order queues overlap lanes, with PSUM reads split
    between DVE and the post-exp-idle scalar engine.

Accuracy (measured vs reference, fixed seed): q/k->e4m3 1.25e-2 dominant,
P/V/O/x/w->bf16 ~+2e-3 => 1.37e-2 < 2e-2 threshold.
"""
import sys

sys.path.insert(0, "/opt/trn_rl_repo")

import numpy as np

DIM, HEADS, B, N = 1024, 16, 2, 2048
D = DIM // HEADS          # 64
HPC = HEADS // 4          # heads per core = 4
NJT = 17                  # j tiles (16 full + void/pad tile)
P = 128

_cache = {}


def _build():
    import concourse.bass as bass
    import concourse.mybir as mybir
    import concourse.tile as tile
    from concourse import bacc

    F32 = mybir.dt.float32
    BF16 = mybir.dt.bfloat16
    F8E4 = mybir.dt.float8e4
    U8 = mybir.dt.uint8
    U16 = mybir.dt.uint16
    Exp = mybir.ActivationFunctionType.Exp
    DR = mybir.MatmulPerfMode.DoubleRow

    nc = bacc.Bacc("TRN2", target_bir_lowering=False, debug=False)
    xT = nc.dram_tensor("xT", [DIM, N], BF16, kind="ExternalInput").ap()
    wqkv = nc.dram_tensor("wqkv", [DIM, 768], BF16, kind="ExternalInput").ap()
    wout = nc.dram_tensor("wout", [256, DIM], BF16, kind="ExternalInput").ap()
    voidk = nc.dram_tensor("voidk", [P, 4], F32, kind="ExternalInput").ap()
    voidv = nc.dram_tensor("voidv", [P, 256], F32, kind="ExternalInput").ap()
    ident_in = nc.dram_tensor("ident_in", [P, P], BF16, kind="ExternalInput").ap()
    y = nc.dram_tensor("y", [N, DIM], F32, kind="ExternalOutput").ap()

    KO = DIM // P  # 8 k-tiles

    with tile.TileContext(nc) as tc:
        with tc.tile_pool(name="persist", bufs=1) as pp, \
             tc.tile_pool(name="work", bufs=1) as wp, \
             tc.tile_pool(name="psum", bufs=1, space="PSUM") as ps, \
             tc.tile_pool(name="loadA", bufs=2) as lp:

            # ---- persistent SBUF tensors ----
            xT_sb = pp.tile([P, KO, N], BF16)
            wqkv_sb = pp.tile([P, KO, 768], BF16)
            wout_sb = pp.tile([P, 2, DIM], BF16)
            qk8 = pp.tile([P, 2, 2, 2, N], F8E4)      # [d, q/k, pair, plane, i]
            va65 = pp.tile([P, 16, 4 * 65], BF16)     # [j, jt, (h,65)]
            kv8 = pp.tile([P, 2, 2], F8E4)            # void-k block diag
            vvb = pp.tile([P, 4, D], F32)             # void v, row-broadcast
            ident = pp.tile([P, P], BF16)

            # ---- PE p-state warm-up: ~6us of dummy matmuls with no input
            # deps, so the tensor engine ramps to full clock during the DMA
            # window and stage A runs at 0.42 ns/row from the start.
            warm = pp.tile([P, 512], BF16)
            nc.gpsimd.memset(warm[:].bitcast(U16), 0)
            for w in range(12):
                wacc = ps.tile([P, 512], F32, tag="spare", name=f"warm_{w}")
                nc.tensor.matmul(wacc[:], warm[:, 0:128], warm[:],
                                 start=True, stop=True)

            # ---- zero planes / pads first: no deps, runs under input DMA ----
            nc.gpsimd.memset(qk8[:, :, :, 1, :].bitcast(U8), 0)
            va_h = va65[:].rearrange("p j (h c) -> p j h c", c=65)
            nc.gpsimd.memset(va_h[:, :, :, 64:65], 1.0)

            # ---- input DMA: column-grouped, arrival order = need order ----
            def dma_wqkv_cols(c0, c1):
                nc.sync.dma_start(
                    wqkv_sb[:, :, c0:c1],
                    wqkv[:, c0:c1].rearrange("(k p) c -> p k c", p=P))

            def dma_xt_sc(sc, k0=0, k1=KO):
                nc.sync.dma_start(
                    xT_sb[:, k0:k1, sc * 512:(sc + 1) * 512],
                    xT[k0 * P:k1 * P, sc * 512:(sc + 1) * 512]
                    .rearrange("(k p) c -> p k c", p=P))

            # host column order: [q-p0 | k-p0 | q-p1 | k-p1 | v]
            dma_wqkv_cols(0, 256)      # pair0 q+k
            dma_xt_sc(0, 0, 4)
            dma_xt_sc(0, 4, 8)
            dma_wqkv_cols(256, 512)    # pair1 q+k
            nc.sync.dma_start(ident[:], ident_in)
            vkt = lp.tile([P, 4], F32, tag="stg")
            nc.sync.dma_start(vkt[:], voidk)
            nc.sync.dma_start(vvb[:].rearrange("p a b -> p (a b)"), voidv)
            dma_wqkv_cols(512, 768)    # v
            dma_xt_sc(1)
            dma_xt_sc(2)
            dma_xt_sc(3)
            nc.sync.dma_start(
                wout_sb[:],
                wout.rearrange("(a p) c -> p a c", p=P))

            # ---- stage A emit helpers ----
            # wqkv column blocks (host order [q-p0 | k-p0 | q-p1 | k-p1 | v])
            FT_COL = {0: 0, 2: 128, 1: 256, 3: 384}

            def emit_aqk_ft(sc, ft, tag="spare", part=None, cache=None):
                """ft 0,1: q pairs; 2,3: k pairs. Evac casts to fp8.
                part 0/1 splits the ko accumulation into two 4-ko halves
                (same PSUM tile, cached) to smooth the per-jt PE load."""
                if part == 1:
                    acc = cache[(sc, ft)]
                else:
                    acc = ps.tile([P, 512], F32, tag=tag,
                                  name=f"aqk_{sc}_{ft}")
                    if cache is not None:
                        cache[(sc, ft)] = acc
                kos = range(KO) if part is None else (
                    range(0, 4) if part == 0 else range(4, KO))
                c0 = FT_COL[ft]
                for ko in kos:
                    nc.tensor.matmul(
                        acc[:],
                        wqkv_sb[:, ko, c0:c0 + P],
                        xT_sb[:, ko, sc * 512:(sc + 1) * 512],
                        start=(ko == 0), stop=(ko == KO - 1),
                    )
                if part != 0:
                    isl = slice(sc * 512, (sc + 1) * 512)
                    qk, pair = (0, ft) if ft < 2 else (1, ft - 2)
                    nc.vector.tensor_copy(qk8[:, qk, pair, 0, isl], acc[:])

            def emit_av_pair(st):
                """av(st), av(st+1) in one spare bank: one start marks the
                bank, the sibling region lands on the pending-zero, and a
                single strided copy casts both into va65."""
                acc = ps.tile([P, 2, 256], F32, tag="spare",
                              name=f"avp_{st}")
                for ko in range(KO):
                    for half in range(2):
                        nc.tensor.matmul(
                            acc[:, half, :],
                            xT_sb[:, ko, (st + half) * P:(st + half + 1) * P],
                            wqkv_sb[:, ko, 512:768],
                            start=(ko == 0 and half == 0), stop=(ko == KO - 1),
                            skip_group_check=True,
                        )
                nc.vector.tensor_copy(
                    va65[:, st:st + 2, :]
                    .rearrange("p j (h c) -> p j h c", c=65)[:, :, :, 0:64],
                    acc[:].rearrange("p j (h c) -> p j h c", c=64))

            def emit_av(st, tag="spare"):
                acc = ps.tile([P, 256], F32, tag=tag, name=f"av_{st}")
                for ko in range(KO):
                    nc.tensor.matmul(
                        acc[:],
                        xT_sb[:, ko, st * P:(st + 1) * P],
                        wqkv_sb[:, ko, 512:768],
                        start=(ko == 0), stop=(ko == KO - 1),
                    )
                nc.vector.tensor_copy(
                    va65[:, st, :].rearrange("p (h c) -> p h c", c=65)[:, :, 0:64],
                    acc[:].rearrange("p (h c) -> p h c", c=64))

            def emit_void_setup():
                nc.vector.tensor_copy(kv8[:].rearrange("p a b -> p (a b)"),
                                      vkt[:])

            # ---- scores: fp8 DoubleRow ----
            def emit_scores_pair(ic, jt, pair):
                isl = slice(ic * 512, (ic + 1) * 512)
                jsl = slice(jt * P, (jt + 1) * P)
                s_pair = ps.tile([P, 1024], F32, tag=f"srot{pair}",
                                 name=f"s_{ic}_{jt}_{pair}")
                for hh in range(2):
                    dsl = slice(hh * D, (hh + 1) * D)
                    nc.tensor.matmul(
                        s_pair[:, hh * 512:(hh + 1) * 512],
                        qk8[dsl, 1, pair, :, jsl],
                        qk8[dsl, 0, pair, :, isl],
                        start=True, stop=True, perf_mode=DR)
                return s_pair

            def emit_scores(ic, jt):
                return [emit_scores_pair(ic, jt, pair) for pair in range(2)]

            # ---- exp + PV ([i,d] orientation) ----
            def emit_exp_pvl(ic, jt, s_cur, pvl, nxt, mid=None):
                """exp(jt) ; scores(nxt) ; [mid()] ; pv(jt)."""
                p_tiles = []
                for pair in range(2):
                    p_pair = wp.tile([P, 1024], BF16, tag=f"pexp{pair}",
                                     bufs=3,
                                     name=f"p_{ic}_{jt}_{pair}")
                    nc.scalar.activation(p_pair[:], s_cur[pair][:], Exp)
                    p_tiles.append(p_pair)
                s_nxt = emit_scores(*nxt) if nxt is not None else None
                if mid is not None:
                    mid()
                # PSUM rule: one start=True per bank; other regions in the
                # bank rely on the bank-wide pending-zero mark (fresh write).
                banks_started = set()
                for pair in range(2):
                    for hh in range(2):
                        h = 2 * pair + hh
                        for ib in range(4):
                            r = ib * 4 + h
                            bank, col = divmod(r, 7)
                            st = jt == 0 and bank not in banks_started
                            if st:
                                banks_started.add(bank)
                            nc.tensor.matmul(
                                pvl[:, bank, col * 65:(col + 1) * 65],
                                p_tiles[pair][:, hh * 512 + ib * P:
                                              hh * 512 + (ib + 1) * P],
                                va65[:, jt, h * 65:(h + 1) * 65],
                                start=st, stop=(jt == 15),
                            )
                return s_nxt

            def emit_svoid(ic):
                """void-key scores in [i, head] orientation + one tiny exp."""
                sv = ps.tile([P, 4, 4], F32, tag="spare", name=f"sv_{ic}")
                for ib in range(4):
                    for pair in range(2):
                        nc.tensor.matmul(
                            sv[:, ib, pair * 2:(pair + 1) * 2],
                            qk8[:, 0, pair, 0,
                                ic * 512 + ib * P:ic * 512 + (ib + 1) * P],
                            kv8[:, pair, :],
                            start=(ib == 0 and pair == 0), stop=False,
                            skip_group_check=True)
                pv = wp.tile([P, 16], F32, tag="pvoid", bufs=2,
                             name=f"pvoid_{ic}")
                nc.scalar.activation(pv[:], sv[:], Exp)
                return pv

            def pv_region(pvs, r):
                bank, col = divmod(r, 7)
                return pvs[:, bank, col * 65:col * 65 + 65]

            def alloc_pvl(ic):
                return ps.tile([P, 3, 512], F32, tag="pvl", name=f"pvl_{ic}")

            # ---- norm / transpose / outproj chain for a finished ic ----
            def emit_pvl_evac(ic, pvl, pv, tail=False):
                """pvl -> SBUF (fat copy, so the next ic's PV never waits;
                per-bank in the tail so the first norms start sooner), then
                the void rank-1 term folded in-place on SBUF:
                pvs_r += v_void[h] * p_void[:, r] ; l += p_void."""
                pvs = wp.tile([P, 3, 512], F32, tag="pvs", bufs=2,
                              name=f"pvs_{ic}")
                if not tail:
                    nc.vector.tensor_copy(pvs[:], pvl[:])
                rl = wp.tile([P, 16], F32, tag="rl", bufs=2, name=f"rl_{ic}")
                for b, (r0, r1) in enumerate(((0, 7), (7, 14), (14, 16))):
                    if tail:
                        nc.vector.tensor_copy(pvs[:, b, :], pvl[:, b, :])
                    for r in range(r0, r1):
                        reg = pv_region(pvs, r)[:, 0:64]
                        nc.vector.scalar_tensor_tensor(
                            reg, vvb[:, r % 4, :], pv[:, r:r + 1], reg,
                            mybir.AluOpType.mult, mybir.AluOpType.add)
                    n = r1 - r0
                    lsl = pvs[:, b, 0:65 * n].rearrange(
                        "p (h c) -> p h c", c=65)[:, :, 64]
                    nc.vector.tensor_tensor(
                        lsl, lsl, pv[:, r0:r1], mybir.AluOpType.add)
                    nc.vector.reciprocal(rl[:, r0:r1], lsl)
                onorm = wp.tile([P, 16, D], BF16, tag="onorm", bufs=2,
                                name=f"onorm_{ic}")
                osb = wp.tile([P, 2, 512], BF16, tag="osb", bufs=2,
                              name=f"osb_{ic}")
                return pvs, rl, onorm, osb

            Copy = mybir.ActivationFunctionType.Copy

            def emit_norm4(ic, ib, pvs, rl, onorm, on_act=False):
                """normalize one i-block. on_act routes the PSUM reads
                through the (post-exp idle) scalar engine."""
                for h in range(4):
                    r = ib * 4 + h
                    if on_act:
                        nc.scalar.activation(
                            onorm[:, r, :], pv_region(pvs, r)[:, 0:64],
                            Copy, scale=rl[:, r:r + 1])
                    else:
                        nc.vector.tensor_scalar(
                            onorm[:, r, :],
                            pv_region(pvs, r)[:, 0:64],
                            rl[:, r:r + 1],
                            None, mybir.AluOpType.mult)

            def emit_tps(ic, ib, onorm, tag):
                sp = ps.tile([P, 2, P], BF16, tag=tag, name=f"tps_{ic}_{ib}")
                for h in range(4):
                    pair, hh = divmod(h, 2)
                    nc.tensor.matmul(
                        sp[hh * D:(hh + 1) * D, pair, :],
                        onorm[:, ib * 4 + h, :],
                        ident[:],
                        # one start per partition-range of the bank (h0/h1);
                        # pair-1 writes land on the pending-zero mark
                        start=(pair == 0), stop=(pair == 1),
                        is_transpose=True)
                return sp

            def emit_tps_evac(ic, ib, sp, osb, on_act=False):
                for pair in range(2):
                    if on_act:
                        nc.scalar.copy(osb[:, pair, ib * P:(ib + 1) * P],
                                       sp[:, pair, :])
                    else:
                        nc.vector.tensor_copy(
                            osb[:, pair, ib * P:(ib + 1) * P], sp[:, pair, :])

            def emit_norm_ib(ic, ib, pvs, rl, onorm, osb, tag="spare",
                             on_act=False):
                emit_norm4(ic, ib, pvs, rl, onorm, on_act)
                sp = emit_tps(ic, ib, onorm, tag)
                emit_tps_evac(ic, ib, sp, osb, on_act)

            def emit_outproj_oc(ic, it, oc, osb, tag, on_act=False):
                ysb = wp.tile([P, 512], F32, tag=f"ysb{oc}", bufs=2,
                              name=f"ysb_{ic}_{it}_{oc}")
                rows = slice(ic * 512 + it * P, ic * 512 + (it + 1) * P)
                yp = ps.tile([P, 512], F32, tag=tag, name=f"y_{ic}_{it}_{oc}")
                for pair in range(2):
                    nc.tensor.matmul(
                        yp[:],
                        osb[:, pair, it * P:(it + 1) * P],
                        wout_sb[:, pair, oc * 512:(oc + 1) * 512],
                        start=(pair == 0), stop=(pair == 1),
                    )
                # PSUM reads go through DVE (or the scalar engine in the
                # post-exp tail; GPSIMD is SBUF-only)
                if on_act:
                    nc.scalar.copy(ysb[:], yp[:])
                else:
                    nc.vector.tensor_copy(ysb[:], yp[:])
                nc.sync.dma_start(y[rows, oc * 512:(oc + 1) * 512], ysb[:])

            def emit_outproj_it(ic, it, osb, tag="spare", on_act=False):
                for oc in range(2):
                    emit_outproj_oc(ic, it, oc, osb, tag,
                                    on_act=(on_act and oc == 1))

            # ---- main schedule ----
            # preamble: stage A for sc0 on the score-psum rotation (no scores
            # contention yet). Each pair's q and k accumulate ko-interleaved
            # into the two banks of one srot tile, so the pair's scores (and
            # the first exp) fire as soon as possible.
            aqkp_acc = {}

            def emit_aqk_half(pair, half):
                """one q-or-k half of the sc0 pair accumulation; the halves
                are emitted interleaved across pairs so the fp8 casts (DVE)
                overlap the other pair's matmuls instead of stalling the
                in-order PE queue."""
                if pair not in aqkp_acc:
                    aqkp_acc[pair] = ps.tile([P, 1024], F32, tag=f"srot{pair}",
                                             name=f"aqkp_{pair}")
                acc = aqkp_acc[pair]
                ft = pair + 2 * half
                c0 = FT_COL[ft]
                for ko in range(KO):
                    nc.tensor.matmul(
                        acc[:, half * 512:(half + 1) * 512],
                        wqkv_sb[:, ko, c0:c0 + P],
                        xT_sb[:, ko, 0:512],
                        start=(ko == 0), stop=(ko == KO - 1),
                    )
                nc.vector.tensor_copy(
                    qk8[:, half, pair, 0, 0:512],
                    acc[:, half * 512:(half + 1) * 512])

            pvl = alloc_pvl(0)
            emit_aqk_half(0, 0)
            emit_aqk_half(0, 1)
            emit_aqk_half(1, 0)
            s00_p0 = emit_scores_pair(0, 0, 0)
            emit_aqk_half(1, 1)
            s00_p1 = emit_scores_pair(0, 0, 1)
            emit_void_setup()
            for st in range(0, 8, 2):
                emit_av_pair(st)

            # stage-A chunks spread across ic0's jt loop (tag "spare", so the
            # score-tile rotation is never delayed). Deadlines: k cols of
            # group q before scores(0, 4q) (emitted at jt 4q-1); av(st)
            # before PV(0, st); q cols of sc1 before scores(1, 0) (at jt16).
            ic0_mid = {
                0: [("aqk", 1, 2)],
                1: [("aqk", 1, 3)],
                2: [("aqk", 1, 0)],
                3: [("aqk", 1, 1)],
                4: [("aqk", 2, 2)],
                5: [("aqk", 2, 3)],
                6: [("aqk", 3, 2)],
                7: [("aqk", 3, 3)],
                8: [("av", 8)],
                9: [("av", 9)],
                10: [("av", 10)],
                11: [("av", 11)],
                12: [("av", 12)],
                13: [("av", 13)],
                14: [("av", 14)],
                15: [("av", 15)],
            }
            def mk_ic0_mid(jt):
                chunks = ic0_mid.get(jt, [])

                def mid():
                    for c in chunks:
                        if c[0] == "av":
                            emit_av(c[1])
                        else:
                            emit_aqk_ft(c[1], c[2])
                return mid

            s_cur = [s00_p0, s00_p1]
            for jt in range(16):
                nxt = (0, jt + 1) if jt < 15 else (1, 0)
                if jt == 13:
                    pv0 = emit_svoid(0)
                s_cur = emit_exp_pvl(0, jt, s_cur, pvl, nxt, mid=mk_ic0_mid(jt))
            

            pvl_prev = pvl
            for ic in range(1, 4):
                chain = {}

                def mk_mid(ic=ic, chain=chain):
                    def mid_jt(jt):
                        def mid():
                            if jt in (1, 2, 3, 4):
                                emit_norm_ib(ic - 1, jt - 1, *chain["t"])
                            elif jt in (5, 6, 7, 8):
                                emit_outproj_it(ic - 1, jt - 5, chain["t"][3])
                            elif jt in (9, 10, 11, 12) and ic < 3:
                                emit_aqk_ft(ic + 1, (jt - 9) // 2,
                                            part=(jt - 9) % 2, cache=chain)
                            elif jt == 14:
                                chain["pv"] = emit_svoid(ic)
                        return mid
                    return mid_jt

                mid_jt = mk_mid()
                pvl = alloc_pvl(ic)
                for jt in range(16):
                    if jt == 15:
                        nxt = (ic + 1, 0) if ic < 3 else None
                    else:
                        nxt = (ic, jt + 1)
                    # pvl evac must precede this ic's first PV accumulation
                    if jt == 0:
                        chain["t"] = emit_pvl_evac(
                            ic - 1, pvl_prev,
                            pv0 if ic == 1 else prev_chain["pv"])
                        s_cur = emit_exp_pvl(ic, jt, s_cur, pvl, nxt)
                    else:
                        s_cur = emit_exp_pvl(ic, jt, s_cur, pvl, nxt,
                                             mid=mid_jt(jt))
                pvl_prev = pvl
                prev_chain = chain

            # tail: last i-chunk's chain in four parallel lanes — after the
            # final exp all four PSUM tag-slots are free (one per lane), work
            # is emitted stage-interleaved so the per-engine in-order queues
            # run the lanes concurrently, and PSUM reads alternate between
            # DVE and the now-idle scalar engine.
            pvs3, rl3, onorm3, osb3 = emit_pvl_evac(
                3, pvl_prev, prev_chain["pv"], tail=True)
            tail_tags = ["srot0", "srot1", "pvl", "spare"]
            for ib in range(4):
                emit_norm4(3, ib, pvs3, rl3, onorm3, on_act=(ib % 2 == 1))
            sps = [emit_tps(3, ib, onorm3, tail_tags[ib]) for ib in range(4)]
            for ib in range(4):
                emit_tps_evac(3, ib, sps[ib], osb3, on_act=(ib % 2 == 1))
                emit_outproj_oc(3, ib, 0, osb3, tail_tags[ib],
                                on_act=(ib % 2 == 0))
            for ib in range(4):
                emit_outproj_oc(3, ib, 1, osb3, tail_tags[ib],
                                on_act=(ib % 2 == 0))

    nc.compile()
    return nc


def _prep_inputs(x, w_qkv, w_out, b_out, void_q, void_k, void_v,
                 attention_trace, temperature_factor):
    """Host-side sharding / layout prep. Returns in_maps for 8 cores."""
    import ml_dtypes
    bf16 = ml_dtypes.bfloat16
    temp = np.maximum(1.0 + np.abs(attention_trace) * temperature_factor,
                      1.0).reshape(HEADS).astype(np.float32)
    scale = (DIM ** -0.5) / temp                       # [16] per head
    ss = np.sqrt(scale)                                # split across q and k
    col_scale = np.repeat(ss, D)                       # [1024]
    wq_scaled = (w_qkv[:, 0:DIM] * col_scale[None, :]).astype(np.float32)
    wk_scaled = (w_qkv[:, DIM:2 * DIM] * col_scale[None, :]).astype(np.float32)
    wv_full = w_qkv[:, 2 * DIM:3 * DIM]

    def wqkv_core(cs):
        # column order [q-p0 | k-p0 | q-p1 | k-p1 | v] -> contiguous DMAs
        q, k = wq_scaled[:, cs], wk_scaled[:, cs]
        return np.concatenate([q[:, 0:128], k[:, 0:128],
                               q[:, 128:256], k[:, 128:256],
                               wv_full[:, cs]], axis=1)
    vk = (void_k.reshape(HEADS, D) * ss[:, None]).astype(np.float32)
    vv = void_v.reshape(HEADS, D).astype(np.float32)
    ident = np.eye(P, dtype=bf16)

    def voidk_core(h0):
        # block-diagonal [128, (pair, col)]: col pair*2+c holds head
        # (h0 + 2*pair + c)'s scaled void key on its own d-rows
        out = np.zeros((P, 4), np.float32)
        for pair in range(2):
            for c in range(2):
                out[c * D:(c + 1) * D, pair * 2 + c] = vk[h0 + 2 * pair + c]
        return out

    def voidv_core(h0):
        # [128, (h, 64)]: void v vectors replicated on every partition
        row = vv[h0:h0 + 4].reshape(256)
        return np.ascontiguousarray(
            np.broadcast_to(row[None, :], (P, 256)).astype(np.float32))

    in_maps = []
    for core in range(8):
        b, hg = divmod(core, 4)
        h0 = hg * HPC
        cs = slice(h0 * D, (h0 + HPC) * D)             # 256 feature cols
        in_maps.append({
            "xT": np.ascontiguousarray(x[b].T.astype(bf16)),
            "wqkv": np.ascontiguousarray(wqkv_core(cs).astype(bf16)),
            "wout": np.ascontiguousarray(w_out[cs, :].astype(bf16)),
            "voidk": voidk_core(h0),
            "voidv": voidv_core(h0),
            "ident_in": ident,
        })
    return in_maps


def _run(in_maps, trace=False):
    from concourse import bass_utils
    if "nc" not in _cache:
        _cache["nc"] = _build()
    return bass_utils.run_bass_kernel_spmd(
        _cache["nc"], in_maps, core_ids=list(range(8)), trace=trace)


def kernel(x, w_qkv, w_out, b_out, void_q, void_k, void_v,
           attention_trace, temperature_factor):
    args = [np.asarray(a, dtype=np.float32) for a in
            (x, w_qkv, w_out, b_out, void_q, void_k, void_v,
             attention_trace, temperature_factor)]
    in_maps = _prep_inputs(*args)
    res = _run(in_maps)
    out = np.zeros((B, N, DIM), np.float32)
    for core in range(8):
        b = core // 4
        out[b] += res.results[core]["y"]
    out += args[3][None, None, :]                      # b_out
    return out
